# revision 12
# baseline (speedup 1.0000x reference)
"""GQA attention (RoPE + causal softmax + out-proj) on 8 TRN2 cores.

Sharding: one core per (batch b, kv-head-group g): 2 batches x 4 kv groups = 8
cores. Each core computes its group's 4 query heads end to end, including the
partial output projection through its 256 rows of wo; the host sums the 4
partial projections per batch (the wo row-shard all-reduce is done on host).

Per-core kernel layout (all "transposed domain": feature dims on partitions,
sequence on the free axis):
  qT [256, S] = wq_g^T x^T, computed as matmul(lhsT=wq_g, rhs=xT); wq columns
  are host-permuted so psum M-tile 0 holds all even (te) rope components
  (4 heads x 32) and M-tile 1 all odd (to). RoPE is then whole-tile vector
  ops against cos/sin tables replicated per head. Same for k (rows 0:64 of
  the packed kv projection; v = rows 64:128).
  scores_T [k 128, q 512] per (head, k-block, q-chunk) = two K=32 row-packed
  matmuls (evens + odds strips) accumulated in psum; exp via ACT (scale=1/8,
  no max subtraction -- scores are O(1) by construction); causality via
  memset + triangular mask multiply on the diagonal blocks only (blocks
  above the diagonal are never computed).
  PV: out_T [65, q 512] accumulated over k-blocks: matmul(lhsT=v_aug[k,65],
  rhs=p[k,q]); v_aug column 64 is ones, so row 64 accumulates the softmax
  denominator. Normalization: reciprocal of that row, broadcast to 64
  partitions via a DRAM bounce DMA, multiplied into attnT.
  Out-proj: matmul(lhsT=attnT[c, s-block], rhs=wo[c, e-chunk]) -> [2048,1024]
  partial, DMA'd out.
All matmuls run as float32r (1 cycle/row at N>=256; fp32 data, reduced
internal precision).
"""

import os
import sys
import types

import numpy as np


def _ensure_axon_hooks_shim():
    """The agent image's antenv package lacks the axon_hooks submodule that
    concourse's trace path imports; install a stub so trace requests degrade
    to no-trace instead of crashing (a real hook can be set into the stub)."""
    try:
        import antenv.axon_hooks  # noqa: F401

        return
    except ImportError:
        pass
    try:
        import antenv
    except ImportError:
        return
    mod = types.ModuleType("antenv.axon_hooks")
    mod._AXON_NTFF_PROFILE_HOOK = None

    def get_axon_ntff_profile_hook():
        return mod._AXON_NTFF_PROFILE_HOOK

    def set_axon_ntff_profile_hook(hook):
        mod._AXON_NTFF_PROFILE_HOOK = hook

    mod.get_axon_ntff_profile_hook = get_axon_ntff_profile_hook
    mod.set_axon_ntff_profile_hook = set_axon_ntff_profile_hook
    sys.modules["antenv.axon_hooks"] = mod
    antenv.axon_hooks = mod


_ensure_axon_hooks_shim()

import concourse.bass as bass
import concourse.bacc as bacc
import concourse.mybir as mybir
import concourse.tile as tile
from concourse.bass_utils import run_bass_kernel_spmd

F32 = mybir.dt.float32
F32R = mybir.dt.float32r
F16 = mybir.dt.float16
AF = mybir.ActivationFunctionType
OP = mybir.AluOpType

B, DIM = 2, 1024
NH, NKV, HD = 16, 4, 64
GH = NH // NKV  # query heads per kv group = 4
S_FULL = 2048
SC = 512  # q chunk width
EXPG = 2  # score psum banks exp'd per ACT call


def build_nc(S=S_FULL, n_cores=8):
    NCH = S // SC
    NKB = S // 128
    KT = DIM // 128  # 8 k-tiles over the model dim

    nc = bacc.Bacc(
        "TRN2", target_bir_lowering=False, debug=False, num_devices=n_cores
    )
    xT = nc.dram_tensor("xT", [DIM, S], F16, kind="ExternalInput").ap()
    wq = nc.dram_tensor("wq", [DIM, 256], F16, kind="ExternalInput").ap()
    wkv = nc.dram_tensor("wkv", [DIM, 128], F16, kind="ExternalInput").ap()
    wo = nc.dram_tensor("wo", [256, DIM], F16, kind="ExternalInput").ap()
    cosr = nc.dram_tensor("cosr", [128, S], F32, kind="ExternalInput").ap()
    sinr = nc.dram_tensor("sinr", [128, S], F32, kind="ExternalInput").ap()
    tri = nc.dram_tensor("tri", [128, 128], F16, kind="ExternalInput").ap()
    ident = nc.dram_tensor("ident", [128, 128], F16, kind="ExternalInput").ap()
    sel = nc.dram_tensor("sel", [64, 128], F16, kind="ExternalInput").ap()
    out = nc.dram_tensor("out", [S, DIM], F32, kind="ExternalOutput").ap()

    xT3 = xT.rearrange("(k p) s -> k p s", p=128)
    wq3 = wq.rearrange("(k p) m -> k p m", p=128)
    wkv3 = wkv.rearrange("(k p) m -> k p m", p=128)
    wo3 = wo.rearrange("(t p) e -> t p e", p=128)

    with tile.TileContext(nc) as tc:
        with tc.tile_pool(name="const", bufs=1) as cp:
            COS = cp.tile([128, S], F32, tag="COS")
            SIN = cp.tile([128, S], F32, tag="SIN")
            WQ = cp.tile([128, KT, 256], F16, tag="WQ")
            WKV = cp.tile([128, KT, 128], F16, tag="WKV")
            WO = cp.tile([128, 2, DIM], F16, tag="WO")
            TRI = cp.tile([128, 128], F16, tag="TRI")
            IDENT = cp.tile([128, 128], F16, tag="IDENT")
            RE = cp.tile([128, S], F16, tag="RE")
            IM = cp.tile([128, S], F16, tag="IM")
            KA4 = cp.tile([128, S], F16, tag="KA4")  # becomes KAB: [KA;KB;KA;KB]
            KB4 = cp.tile([128, S], F16, tag="KB4")
            REIM0 = cp.tile([128, S], F16, tag="REIM0")
            REIM1 = cp.tile([128, S], F16, tag="REIM1")
            SEL = cp.tile([64, 128], F16, tag="SEL")
            VT = cp.tile([64, S], F16, tag="VT")
            VAUG = cp.tile([128, NKB, 128], F16, tag="VAUG")
            AT0 = cp.tile([128, S], F16, tag="AT0")
            AT1 = cp.tile([128, S], F16, tag="AT1")

            nc.sync.dma_start(COS[:], cosr)
            nc.sync.dma_start(SIN[:], sinr)
            nc.sync.dma_start(TRI[:], tri)
            nc.sync.dma_start(IDENT[:], ident)
            nc.sync.dma_start(SEL[:], sel)
            nc.sync.dma_start(
                WQ[:], wq.rearrange("(k p) m -> p k m", p=128)
            )
            nc.sync.dma_start(
                WKV[:], wkv.rearrange("(k p) m -> p k m", p=128)
            )
            nc.sync.dma_start(
                WO[:], wo.rearrange("(t p) e -> p t e", p=128)
            )
            ONES = cp.tile([1, 64], F16, tag="ONES")
            nc.vector.memset(ONES[:], 1.0)
            nc.vector.memset(VAUG[:], 1.0)

            # ---- Phase A: projections + rope + v transpose ----
            with (
                tc.tile_pool(name="apsum", bufs=2, space="PSUM") as aps,
                tc.tile_pool(name="xt", bufs=3) as xp,
                tc.tile_pool(name="rt", bufs=3) as rt,
            ):
                for qc in range(NCH):
                    sl = slice(qc * SC, (qc + 1) * SC)
                    q0 = aps.tile([128, SC], F32, tag="q0")
                    q1 = aps.tile([128, SC], F32, tag="q1")
                    kv = aps.tile([128, SC], F32, tag="kv")
                    for kt in range(KT):
                        xt_t = xp.tile([128, SC], F16, tag="xt")
                        nc.sync.dma_start(xt_t[:], xT3[kt, :, sl])
                        st, sp = kt == 0, kt == KT - 1
                        nc.tensor.matmul(
                            q0[:], WQ[:, kt, 0:128],
                            xt_t[:], start=st, stop=sp,
                        )
                        nc.tensor.matmul(
                            q1[:], WQ[:, kt, 128:256],
                            xt_t[:], start=st, stop=sp,
                        )
                        nc.tensor.matmul(
                            kv[:], WKV[:, kt, :],
                            xt_t[:], start=st, stop=sp,
                        )
                    # rope q: RE = te*cos - to*sin ; IM = te*sin + to*cos
                    t1 = rt.tile([128, SC], F32, tag="t1")
                    t2 = rt.tile([128, SC], F32, tag="t2")
                    nc.vector.tensor_tensor(t1[:], q0[:], COS[:, sl], OP.mult)
                    nc.vector.tensor_tensor(t2[:], q1[:], SIN[:, sl], OP.mult)
                    nc.vector.tensor_tensor(RE[:, sl], t1[:], t2[:], OP.subtract)
                    t3 = rt.tile([128, SC], F32, tag="t3")
                    t4 = rt.tile([128, SC], F32, tag="t4")
                    nc.vector.tensor_tensor(t3[:], q0[:], SIN[:, sl], OP.mult)
                    nc.vector.tensor_tensor(t4[:], q1[:], COS[:, sl], OP.mult)
                    nc.vector.tensor_tensor(IM[:, sl], t3[:], t4[:], OP.add)
                    # rope k: kv rows 0:32 = evens, 32:64 = odds
                    u1 = rt.tile([32, SC], F32, tag="u1")
                    u2 = rt.tile([32, SC], F32, tag="u2")
                    nc.vector.tensor_tensor(u1[:], kv[0:32, :], COS[0:32, sl], OP.mult)
                    nc.vector.tensor_tensor(u2[:], kv[32:64, :], SIN[0:32, sl], OP.mult)
                    nc.vector.tensor_tensor(KA4[0:32, sl], u1[:], u2[:], OP.subtract)
                    u3 = rt.tile([32, SC], F32, tag="u3")
                    u4 = rt.tile([32, SC], F32, tag="u4")
                    nc.vector.tensor_tensor(u3[:], kv[0:32, :], SIN[0:32, sl], OP.mult)
                    nc.vector.tensor_tensor(u4[:], kv[32:64, :], COS[0:32, sl], OP.mult)
                    nc.vector.tensor_tensor(KB4[0:32, sl], u3[:], u4[:], OP.add)
                    # v
                    nc.vector.tensor_copy(VT[0:64, sl], kv[64:128, :])
                # build KAB = [KA;KB;KA;KB] in KA4, and the per-head-pair
                # interleaved [RE_h;IM_h] rhs tiles (SBUF->SBUF DMA)
                nc.sync.dma_start(KA4[32:64, :], KB4[0:32, :])
                nc.sync.dma_start(KA4[64:96, :], KA4[0:32, :])
                nc.sync.dma_start(KA4[96:128, :], KB4[0:32, :])
                for t, RT in enumerate((REIM0, REIM1)):
                    for half in range(2):
                        h = 2 * t + half
                        rq = slice(32 * h, 32 * h + 32)
                        nc.sync.dma_start(RT[64 * half : 64 * half + 32, :], RE[rq, :])
                        nc.sync.dma_start(RT[64 * half + 32 : 64 * half + 64, :], IM[rq, :])
                # v transpose: vT [64, S] -> v_aug [k, 65] blocks
                for kb in range(NKB):
                    vp = aps.tile([128, 64], F16, tag="vp")
                    nc.tensor.transpose(
                        vp[:],
                        VT[0:64, kb * 128 : (kb + 1) * 128],
                        IDENT[0:64, 0:64],
                    )
                    nc.vector.tensor_copy(VAUG[:, kb, 0:HD], vp[:])

            # ---- Phase B: attention ----
            with (
                tc.tile_pool(name="bpsum", bufs=2, space="PSUM") as bps,
                tc.tile_pool(name="opsum", bufs=2, space="PSUM") as ops,
                tc.tile_pool(name="pp", bufs=3) as pp,
                tc.tile_pool(name="np_", bufs=2) as npo,
            ):
                for qc in range(NCH):
                    qsl = slice(qc * SC, (qc + 1) * SC)
                    nkb = 4 * qc + 4
                    for pr in range(2):  # head pairs (0,1) and (2,3)
                        RT = (REIM0, REIM1)[pr]
                        ot0 = ops.tile([128, SC], F32, tag="ot0")
                        ot1 = ops.tile([128, SC], F32, tag="ot1")
                        ots = (ot0, ot1)
                        for kb in range(nkb):
                            ksl = slice(kb * 128, (kb + 1) * 128)
                            sc_ps = bps.tile([128, 2, SC], F32, tag="sc")
                            for j in range(2):  # head-in-pair; strips alternate
                                rs = slice(64 * j, 64 * j + 64)
                                nc.tensor.matmul(
                                    sc_ps[:, j, :],
                                    KA4[rs, ksl],
                                    RT[rs, qsl],
                                    start=True, stop=True,
                                    tile_position=(64 * j, 0),
                                )
                            p_sb = pp.tile([128, 2, SC], F16, tag="p")
                            nc.scalar.activation(
                                p_sb[:], sc_ps[:], AF.Exp, scale=0.125
                            )
                            jj = kb - 4 * qc
                            for j in range(2):
                                if jj >= 0:
                                    if jj > 0:
                                        nc.vector.memset(
                                            p_sb[:, j, 0 : jj * 128], 0.0
                                        )
                                    dsl = slice(jj * 128, (jj + 1) * 128)
                                    nc.vector.tensor_tensor(
                                        p_sb[:, j, dsl], p_sb[:, j, dsl],
                                        TRI[:], OP.mult,
                                    )
                                nc.tensor.matmul(
                                    ots[j][:],
                                    VAUG[:, kb, :],
                                    p_sb[:, j, :],
                                    start=(kb == 0), stop=(kb == nkb - 1),
                                )
                        # normalize both heads of the pair: denom rows ->
                        # one batched reciprocal -> SEL-matmul broadcast
                        den = npo.tile([64, SC], F32, tag="den")
                        nc.vector.memset(den[:], 1.0)
                        nc.vector.tensor_copy(den[0:1, :], ot0[64:65, :])
                        nc.vector.tensor_copy(den[32:33, :], ot1[64:65, :])
                        rec = npo.tile([64, SC], F16, tag="rec")
                        with nc.allow_low_precision(
                            reason="fp16 softmax denominators"
                        ):
                            nc.vector.reciprocal(rec[:], den[:])
                        rbc_ps = bps.tile([128, 2, SC], F32, tag="sc")
                        nc.tensor.matmul(
                            rbc_ps[:, 0, :], SEL[:], rec[:], start=True, stop=True
                        )
                        rbc_sb = npo.tile([128, SC], F32, tag="rbc_sb")
                        nc.vector.tensor_copy(rbc_sb[:], rbc_ps[:, 0, :])
                        att = (AT0, AT1)[pr]
                        nc.vector.tensor_tensor(
                            att[0:64, qsl], ot0[0:64, :], rbc_sb[0:64, :], OP.mult
                        )
                        nc.vector.tensor_tensor(
                            att[64:128, qsl], ot1[0:64, :], rbc_sb[64:128, :], OP.mult
                        )

            # ---- Phase C: output projection ----
            with (
                tc.tile_pool(name="cpsum", bufs=2, space="PSUM") as cps,
                tc.tile_pool(name="op", bufs=3) as op_pool,
            ):
                for sb_i in range(S // 128):
                    ssl = slice(sb_i * 128, (sb_i + 1) * 128)
                    for ec in range(DIM // 512):
                        esl = slice(ec * 512, (ec + 1) * 512)
                        o_ps = cps.tile([128, 512], F32, tag="o")
                        for t in range(2):
                            att = (AT0, AT1)[t]
                            nc.tensor.matmul(
                                o_ps[:],
                                att[:, ssl],
                                WO[:, t, esl],
                                start=(t == 0), stop=(t == 1),
                            )
                        ost = op_pool.tile([128, 512], F32, tag="ost")
                        nc.any.tensor_copy(ost[:], o_ps[:])
                        nc.sync.dma_start(out[ssl, esl], ost[:])

    nc.compile()
    return nc


# host-side column permutations: all rope-even dims first, then all odds
_PERM256 = np.array(
    [64 * h + 2 * i for h in range(4) for i in range(32)]
    + [64 * h + 2 * i + 1 for h in range(4) for i in range(32)]
)
_PERM64 = np.array([2 * i for i in range(32)] + [2 * i + 1 for i in range(32)])

_cache = {}


def make_in_maps(x, cos, sin, wq, wk, wv, wo, n_groups=4):
    S = x.shape[1]
    cos_r = np.ascontiguousarray(np.tile(cos.T, (4, 1)), dtype=np.float32)
    sin_r = np.ascontiguousarray(np.tile(sin.T, (4, 1)), dtype=np.float32)
    tri = np.triu(np.ones((128, 128), dtype=np.float16))
    ident = np.eye(128, dtype=np.float16)
    sel = np.zeros((64, 128), dtype=np.float16)
    sel[0, 0:64] = 1.0
    sel[32, 64:128] = 1.0
    xTs = [np.ascontiguousarray(x[b].T.astype(np.float16)) for b in range(x.shape[0])]
    in_maps = []
    for c in range(x.shape[0] * n_groups):
        b, g = divmod(c, n_groups)
        wq_c = np.ascontiguousarray(wq[:, 256 * g + _PERM256].astype(np.float16))
        wk_c = wk[:, 64 * g + _PERM64]
        wv_c = wv[:, 64 * g : 64 * (g + 1)]
        wkv_c = np.ascontiguousarray(
            np.concatenate([wk_c, wv_c], axis=1), dtype=np.float16
        )
        wo_c = np.ascontiguousarray(wo[256 * g : 256 * (g + 1), :].astype(np.float16))
        in_maps.append(
            {
                "xT": xTs[b],
                "wq": wq_c,
                "wkv": wkv_c,
                "wo": wo_c,
                "cosr": cos_r,
                "sinr": sin_r,
                "tri": tri,
                "ident": ident,
                "sel": sel,
            }
        )
    return in_maps


def kernel(x, cos, sin, mask, wq, wk, wv, wo):
    x = np.asarray(x, dtype=np.float32)
    cos = np.asarray(cos, dtype=np.float32)
    sin = np.asarray(sin, dtype=np.float32)
    wq = np.asarray(wq, dtype=np.float32)
    wk = np.asarray(wk, dtype=np.float32)
    wv = np.asarray(wv, dtype=np.float32)
    wo = np.asarray(wo, dtype=np.float32)

    if "nc" not in _cache:
        _cache["nc"] = build_nc(S=x.shape[1], n_cores=8)
    nc = _cache["nc"]
    in_maps = make_in_maps(x, cos, sin, wq, wk, wv, wo)
    res = run_bass_kernel_spmd(nc, in_maps, list(range(8)))
    _cache["last"] = res
    outs = [r["out"] for r in res.results]
    final = np.stack(
        [outs[0] + outs[1] + outs[2] + outs[3], outs[4] + outs[5] + outs[6] + outs[7]],
        axis=0,
    )
    return final.astype(np.float32)


# revision 13
# speedup vs baseline: 1.0134x; 1.0134x over previous
"""GQA attention (RoPE + causal softmax + out-proj) on 8 TRN2 cores.

Sharding: one core per (batch b, kv-head-group g): 2 batches x 4 kv groups = 8
cores. Each core computes its group's 4 query heads end to end, including the
partial output projection through its 256 rows of wo; the host sums the 4
partial projections per batch (the wo row-shard all-reduce is done on host).

Per-core kernel layout (all "transposed domain": feature dims on partitions,
sequence on the free axis):
  qT [256, S] = wq_g^T x^T, computed as matmul(lhsT=wq_g, rhs=xT); wq columns
  are host-permuted so psum M-tile 0 holds all even (te) rope components
  (4 heads x 32) and M-tile 1 all odd (to). RoPE is then whole-tile vector
  ops against cos/sin tables replicated per head. Same for k (rows 0:64 of
  the packed kv projection; v = rows 64:128).
  scores_T [k 128, q 512] per (head, k-block, q-chunk) = two K=32 row-packed
  matmuls (evens + odds strips) accumulated in psum; exp via ACT (scale=1/8,
  no max subtraction -- scores are O(1) by construction); causality via
  memset + triangular mask multiply on the diagonal blocks only (blocks
  above the diagonal are never computed).
  PV: out_T [65, q 512] accumulated over k-blocks: matmul(lhsT=v_aug[k,65],
  rhs=p[k,q]); v_aug column 64 is ones, so row 64 accumulates the softmax
  denominator. Normalization: reciprocal of that row, broadcast to 64
  partitions via a DRAM bounce DMA, multiplied into attnT.
  Out-proj: matmul(lhsT=attnT[c, s-block], rhs=wo[c, e-chunk]) -> [2048,1024]
  partial, DMA'd out.
All matmuls run as float32r (1 cycle/row at N>=256; fp32 data, reduced
internal precision).
"""

import os
import sys
import types

import numpy as np


def _ensure_axon_hooks_shim():
    """The agent image's antenv package lacks the axon_hooks submodule that
    concourse's trace path imports; install a stub so trace requests degrade
    to no-trace instead of crashing (a real hook can be set into the stub)."""
    try:
        import antenv.axon_hooks  # noqa: F401

        return
    except ImportError:
        pass
    try:
        import antenv
    except ImportError:
        return
    mod = types.ModuleType("antenv.axon_hooks")
    mod._AXON_NTFF_PROFILE_HOOK = None

    def get_axon_ntff_profile_hook():
        return mod._AXON_NTFF_PROFILE_HOOK

    def set_axon_ntff_profile_hook(hook):
        mod._AXON_NTFF_PROFILE_HOOK = hook

    mod.get_axon_ntff_profile_hook = get_axon_ntff_profile_hook
    mod.set_axon_ntff_profile_hook = set_axon_ntff_profile_hook
    sys.modules["antenv.axon_hooks"] = mod
    antenv.axon_hooks = mod


_ensure_axon_hooks_shim()

import concourse.bass as bass
import concourse.bacc as bacc
import concourse.mybir as mybir
import concourse.tile as tile
from concourse.bass_utils import run_bass_kernel_spmd

F32 = mybir.dt.float32
F32R = mybir.dt.float32r
F16 = mybir.dt.float16
AF = mybir.ActivationFunctionType
OP = mybir.AluOpType

B, DIM = 2, 1024
NH, NKV, HD = 16, 4, 64
GH = NH // NKV  # query heads per kv group = 4
S_FULL = 2048
SC = 512  # q chunk width
EXPG = 2  # score psum banks exp'd per ACT call


def build_nc(S=S_FULL, n_cores=8):
    NCH = S // SC
    NKB = S // 128
    KT = DIM // 128  # 8 k-tiles over the model dim

    nc = bacc.Bacc(
        "TRN2", target_bir_lowering=False, debug=False, num_devices=n_cores
    )
    xT = nc.dram_tensor("xT", [DIM, S], F16, kind="ExternalInput").ap()
    wq = nc.dram_tensor("wq", [DIM, 256], F16, kind="ExternalInput").ap()
    wkv = nc.dram_tensor("wkv", [DIM, 128], F16, kind="ExternalInput").ap()
    wo = nc.dram_tensor("wo", [256, DIM], F16, kind="ExternalInput").ap()
    cosr = nc.dram_tensor("cosr", [128, S], F32, kind="ExternalInput").ap()
    sinr = nc.dram_tensor("sinr", [128, S], F32, kind="ExternalInput").ap()
    tri = nc.dram_tensor("tri", [128, 128], F16, kind="ExternalInput").ap()
    ident = nc.dram_tensor("ident", [128, 128], F16, kind="ExternalInput").ap()
    sel = nc.dram_tensor("sel", [64, 128], F16, kind="ExternalInput").ap()
    ntri = nc.dram_tensor("ntri", [128, 896], F16, kind="ExternalInput").ap()
    out = nc.dram_tensor("out", [S, DIM], F32, kind="ExternalOutput").ap()

    xT3 = xT.rearrange("(k p) s -> k p s", p=128)
    wq3 = wq.rearrange("(k p) m -> k p m", p=128)
    wkv3 = wkv.rearrange("(k p) m -> k p m", p=128)
    wo3 = wo.rearrange("(t p) e -> t p e", p=128)

    with tile.TileContext(nc) as tc:
        with tc.tile_pool(name="const", bufs=1) as cp:
            COS = cp.tile([128, S], F32, tag="COS")
            SIN = cp.tile([128, S], F32, tag="SIN")
            WQ = cp.tile([128, KT, 256], F16, tag="WQ")
            WKV = cp.tile([128, KT, 128], F16, tag="WKV")
            WO = cp.tile([128, 2, DIM], F16, tag="WO")
            TRI = cp.tile([128, 128], F16, tag="TRI")
            IDENT = cp.tile([128, 128], F16, tag="IDENT")
            RE = cp.tile([128, S], F16, tag="RE")
            IM = cp.tile([128, S], F16, tag="IM")
            KA4 = cp.tile([128, S], F16, tag="KA4")  # becomes KAB: [KA;KB;KA;KB]
            KB4 = cp.tile([128, S], F16, tag="KB4")
            REIM0 = cp.tile([128, S], F16, tag="REIM0")
            REIM1 = cp.tile([128, S], F16, tag="REIM1")
            SEL = cp.tile([64, 128], F16, tag="SEL")
            NTRI = cp.tile([128, 896], F16, tag="NTRI")
            VT = cp.tile([64, S], F16, tag="VT")
            VAUG = cp.tile([128, NKB, 128], F16, tag="VAUG")
            AT0 = cp.tile([128, S], F16, tag="AT0")
            AT1 = cp.tile([128, S], F16, tag="AT1")

            nc.sync.dma_start(COS[:], cosr)
            nc.sync.dma_start(SIN[:], sinr)
            nc.sync.dma_start(TRI[:], tri)
            nc.sync.dma_start(IDENT[:], ident)
            nc.sync.dma_start(SEL[:], sel)
            nc.sync.dma_start(NTRI[:], ntri)
            nc.sync.dma_start(
                WQ[:], wq.rearrange("(k p) m -> p k m", p=128)
            )
            nc.sync.dma_start(
                WKV[:], wkv.rearrange("(k p) m -> p k m", p=128)
            )
            nc.sync.dma_start(
                WO[:], wo.rearrange("(t p) e -> p t e", p=128)
            )
            ONES = cp.tile([1, 64], F16, tag="ONES")
            nc.vector.memset(ONES[:], 1.0)
            nc.vector.memset(VAUG[:], 1.0)

            # ---- Phase A: projections + rope + v transpose ----
            with (
                tc.tile_pool(name="apsum", bufs=2, space="PSUM") as aps,
                tc.tile_pool(name="xt", bufs=3) as xp,
                tc.tile_pool(name="rt", bufs=3) as rt,
            ):
                for qc in range(NCH):
                    sl = slice(qc * SC, (qc + 1) * SC)
                    q0 = aps.tile([128, SC], F32, tag="q0")
                    q1 = aps.tile([128, SC], F32, tag="q1")
                    kv = aps.tile([128, SC], F32, tag="kv")
                    for kt in range(KT):
                        xt_t = xp.tile([128, SC], F16, tag="xt")
                        nc.sync.dma_start(xt_t[:], xT3[kt, :, sl])
                        st, sp = kt == 0, kt == KT - 1
                        nc.tensor.matmul(
                            q0[:], WQ[:, kt, 0:128],
                            xt_t[:], start=st, stop=sp,
                        )
                        nc.tensor.matmul(
                            q1[:], WQ[:, kt, 128:256],
                            xt_t[:], start=st, stop=sp,
                        )
                        nc.tensor.matmul(
                            kv[:], WKV[:, kt, :],
                            xt_t[:], start=st, stop=sp,
                        )
                    # rope q: RE = te*cos - to*sin ; IM = te*sin + to*cos
                    t1 = rt.tile([128, SC], F32, tag="t1")
                    t2 = rt.tile([128, SC], F32, tag="t2")
                    nc.vector.tensor_tensor(t1[:], q0[:], COS[:, sl], OP.mult)
                    nc.vector.tensor_tensor(t2[:], q1[:], SIN[:, sl], OP.mult)
                    nc.vector.tensor_tensor(RE[:, sl], t1[:], t2[:], OP.subtract)
                    t3 = rt.tile([128, SC], F32, tag="t3")
                    t4 = rt.tile([128, SC], F32, tag="t4")
                    nc.vector.tensor_tensor(t3[:], q0[:], SIN[:, sl], OP.mult)
                    nc.vector.tensor_tensor(t4[:], q1[:], COS[:, sl], OP.mult)
                    nc.vector.tensor_tensor(IM[:, sl], t3[:], t4[:], OP.add)
                    # rope k: kv rows 0:32 = evens, 32:64 = odds
                    u1 = rt.tile([32, SC], F32, tag="u1")
                    u2 = rt.tile([32, SC], F32, tag="u2")
                    nc.vector.tensor_tensor(u1[:], kv[0:32, :], COS[0:32, sl], OP.mult)
                    nc.vector.tensor_tensor(u2[:], kv[32:64, :], SIN[0:32, sl], OP.mult)
                    nc.vector.tensor_tensor(KA4[0:32, sl], u1[:], u2[:], OP.subtract)
                    u3 = rt.tile([32, SC], F32, tag="u3")
                    u4 = rt.tile([32, SC], F32, tag="u4")
                    nc.vector.tensor_tensor(u3[:], kv[0:32, :], SIN[0:32, sl], OP.mult)
                    nc.vector.tensor_tensor(u4[:], kv[32:64, :], COS[0:32, sl], OP.mult)
                    nc.vector.tensor_tensor(KB4[0:32, sl], u3[:], u4[:], OP.add)
                    # v
                    nc.vector.tensor_copy(VT[0:64, sl], kv[64:128, :])
                # build KAB = [KA;KB;KA;KB] in KA4, and the per-head-pair
                # interleaved [RE_h;IM_h] rhs tiles (SBUF->SBUF DMA)
                nc.sync.dma_start(KA4[32:64, :], KB4[0:32, :])
                nc.sync.dma_start(KA4[64:96, :], KA4[0:32, :])
                nc.sync.dma_start(KA4[96:128, :], KB4[0:32, :])
                for t, RT in enumerate((REIM0, REIM1)):
                    for half in range(2):
                        h = 2 * t + half
                        rq = slice(32 * h, 32 * h + 32)
                        nc.sync.dma_start(RT[64 * half : 64 * half + 32, :], RE[rq, :])
                        nc.sync.dma_start(RT[64 * half + 32 : 64 * half + 64, :], IM[rq, :])
                # v transpose: vT [64, S] -> v_aug [k, 65] blocks
                for kb in range(NKB):
                    vp = aps.tile([128, 64], F16, tag="vp")
                    nc.tensor.transpose(
                        vp[:],
                        VT[0:64, kb * 128 : (kb + 1) * 128],
                        IDENT[0:64, 0:64],
                    )
                    nc.vector.tensor_copy(VAUG[:, kb, 0:HD], vp[:])

            # ---- Phase B: attention ----
            with (
                tc.tile_pool(name="bpsum", bufs=3, space="PSUM") as bps,
                tc.tile_pool(name="opsum", bufs=1, space="PSUM") as ops,
                tc.tile_pool(name="pp", bufs=4) as pp,
                tc.tile_pool(name="np_", bufs=2) as npo,
            ):
                for qc in range(NCH):
                    qsl = slice(qc * SC, (qc + 1) * SC)
                    nkb = 4 * qc + 4
                    for pr in range(2):  # head pairs (0,1) and (2,3)
                        RT = (REIM0, REIM1)[pr]
                        ot0 = ops.tile([128, SC], F32, tag="ot0")
                        ot1 = ops.tile([128, SC], F32, tag="ot1")
                        ots = (ot0, ot1)
                        def emit_pv(kb, p_sb):
                            for j in range(2):
                                nc.tensor.matmul(
                                    ots[j][:],
                                    VAUG[:, kb, :],
                                    p_sb[:, j, :],
                                    start=(kb == 0), stop=(kb == nkb - 1),
                                )

                        staged = []
                        for kb in range(nkb):
                            ksl = slice(kb * 128, (kb + 1) * 128)
                            jj = kb - 4 * qc
                            sc_ps = bps.tile([128, 2, SC], F32, tag="sc")
                            for j in range(2):  # head-in-pair; strips alternate
                                rs = slice(64 * j, 64 * j + 64)
                                nc.tensor.matmul(
                                    sc_ps[:, j, :],
                                    KA4[rs, ksl],
                                    RT[rs, qsl],
                                    start=True, stop=(jj < 0),
                                    tile_position=(64 * j, 0),
                                )
                            if jj >= 0:
                                # causal mask: accumulate a -1e4 upper strip
                                nsl = slice(384 - 128 * jj, 896 - 128 * jj)
                                for j in range(2):
                                    nc.tensor.matmul(
                                        sc_ps[:, j, :],
                                        IDENT[:],
                                        NTRI[:, nsl],
                                        start=False, stop=True,
                                    )
                            p_sb = pp.tile([128, 2, SC], F16, tag="p")
                            nc.scalar.activation(
                                p_sb[:], sc_ps[:], AF.Exp, scale=0.125
                            )
                            staged.append((kb, p_sb))
                            if len(staged) > 2:
                                emit_pv(*staged.pop(0))
                        for it in staged:
                            emit_pv(*it)
                                                # normalize both heads of the pair: denom rows ->
                        # one batched reciprocal -> SEL-matmul broadcast
                        den = npo.tile([64, SC], F32, tag="den")
                        nc.vector.memset(den[:], 1.0)
                        nc.vector.tensor_copy(den[0:1, :], ot0[64:65, :])
                        nc.vector.tensor_copy(den[32:33, :], ot1[64:65, :])
                        rec = npo.tile([64, SC], F16, tag="rec")
                        with nc.allow_low_precision(
                            reason="fp16 softmax denominators"
                        ):
                            nc.vector.reciprocal(rec[:], den[:])
                        rbc_ps = bps.tile([128, 2, SC], F32, tag="sc")
                        nc.tensor.matmul(
                            rbc_ps[:, 0, :], SEL[:], rec[:], start=True, stop=True
                        )
                        rbc_sb = npo.tile([128, SC], F32, tag="rbc_sb")
                        nc.vector.tensor_copy(rbc_sb[:], rbc_ps[:, 0, :])
                        att = (AT0, AT1)[pr]
                        nc.vector.tensor_tensor(
                            att[0:64, qsl], ot0[0:64, :], rbc_sb[0:64, :], OP.mult
                        )
                        nc.vector.tensor_tensor(
                            att[64:128, qsl], ot1[0:64, :], rbc_sb[64:128, :], OP.mult
                        )

            # ---- Phase C: output projection ----
            with (
                tc.tile_pool(name="cpsum", bufs=2, space="PSUM") as cps,
                tc.tile_pool(name="op", bufs=3) as op_pool,
            ):
                for sb_i in range(S // 128):
                    ssl = slice(sb_i * 128, (sb_i + 1) * 128)
                    for ec in range(DIM // 512):
                        esl = slice(ec * 512, (ec + 1) * 512)
                        o_ps = cps.tile([128, 512], F32, tag="o")
                        for t in range(2):
                            att = (AT0, AT1)[t]
                            nc.tensor.matmul(
                                o_ps[:],
                                att[:, ssl],
                                WO[:, t, esl],
                                start=(t == 0), stop=(t == 1),
                            )
                        ost = op_pool.tile([128, 512], F32, tag="ost")
                        nc.any.tensor_copy(ost[:], o_ps[:])
                        nc.sync.dma_start(out[ssl, esl], ost[:])

    nc.compile()
    return nc


# host-side column permutations: all rope-even dims first, then all odds
_PERM256 = np.array(
    [64 * h + 2 * i for h in range(4) for i in range(32)]
    + [64 * h + 2 * i + 1 for h in range(4) for i in range(32)]
)
_PERM64 = np.array([2 * i for i in range(32)] + [2 * i + 1 for i in range(32)])

_cache = {}


def make_in_maps(x, cos, sin, wq, wk, wv, wo, n_groups=4):
    S = x.shape[1]
    cos_r = np.ascontiguousarray(np.tile(cos.T, (4, 1)), dtype=np.float32)
    sin_r = np.ascontiguousarray(np.tile(sin.T, (4, 1)), dtype=np.float32)
    tri = np.triu(np.ones((128, 128), dtype=np.float16))
    ident = np.eye(128, dtype=np.float16)
    sel = np.zeros((64, 128), dtype=np.float16)
    sel[0, 0:64] = 1.0
    sel[32, 64:128] = 1.0
    uu, pp_ = np.meshgrid(np.arange(896), np.arange(128))
    ntri = np.where(uu < pp_ + 384, np.float16(-10000.0), np.float16(0.0)).astype(
        np.float16
    )
    xTs = [np.ascontiguousarray(x[b].T.astype(np.float16)) for b in range(x.shape[0])]
    in_maps = []
    for c in range(x.shape[0] * n_groups):
        b, g = divmod(c, n_groups)
        wq_c = np.ascontiguousarray(wq[:, 256 * g + _PERM256].astype(np.float16))
        wk_c = wk[:, 64 * g + _PERM64]
        wv_c = wv[:, 64 * g : 64 * (g + 1)]
        wkv_c = np.ascontiguousarray(
            np.concatenate([wk_c, wv_c], axis=1), dtype=np.float16
        )
        wo_c = np.ascontiguousarray(wo[256 * g : 256 * (g + 1), :].astype(np.float16))
        in_maps.append(
            {
                "xT": xTs[b],
                "wq": wq_c,
                "wkv": wkv_c,
                "wo": wo_c,
                "cosr": cos_r,
                "sinr": sin_r,
                "tri": tri,
                "ident": ident,
                "sel": sel,
                "ntri": ntri,
            }
        )
    return in_maps


def kernel(x, cos, sin, mask, wq, wk, wv, wo):
    x = np.asarray(x, dtype=np.float32)
    cos = np.asarray(cos, dtype=np.float32)
    sin = np.asarray(sin, dtype=np.float32)
    wq = np.asarray(wq, dtype=np.float32)
    wk = np.asarray(wk, dtype=np.float32)
    wv = np.asarray(wv, dtype=np.float32)
    wo = np.asarray(wo, dtype=np.float32)

    if "nc" not in _cache:
        _cache["nc"] = build_nc(S=x.shape[1], n_cores=8)
    nc = _cache["nc"]
    in_maps = make_in_maps(x, cos, sin, wq, wk, wv, wo)
    res = run_bass_kernel_spmd(nc, in_maps, list(range(8)))
    _cache["last"] = res
    outs = [r["out"] for r in res.results]
    final = np.stack(
        [outs[0] + outs[1] + outs[2] + outs[3], outs[4] + outs[5] + outs[6] + outs[7]],
        axis=0,
    )
    return final.astype(np.float32)


# revision 14
# speedup vs baseline: 1.0172x; 1.0037x over previous
"""GQA attention (RoPE + causal softmax + out-proj) on 8 TRN2 cores.

Sharding: one core per (batch b, kv-head-group g): 2 batches x 4 kv groups = 8
cores. Each core computes its group's 4 query heads end to end, including the
partial output projection through its 256 rows of wo; the host sums the 4
partial projections per batch (the wo row-shard all-reduce is done on host).

Per-core kernel layout (all "transposed domain": feature dims on partitions,
sequence on the free axis):
  qT [256, S] = wq_g^T x^T, computed as matmul(lhsT=wq_g, rhs=xT); wq columns
  are host-permuted so psum M-tile 0 holds all even (te) rope components
  (4 heads x 32) and M-tile 1 all odd (to). RoPE is then whole-tile vector
  ops against cos/sin tables replicated per head. Same for k (rows 0:64 of
  the packed kv projection; v = rows 64:128).
  scores_T [k 128, q 512] per (head, k-block, q-chunk) = two K=32 row-packed
  matmuls (evens + odds strips) accumulated in psum; exp via ACT (scale=1/8,
  no max subtraction -- scores are O(1) by construction); causality via
  memset + triangular mask multiply on the diagonal blocks only (blocks
  above the diagonal are never computed).
  PV: out_T [65, q 512] accumulated over k-blocks: matmul(lhsT=v_aug[k,65],
  rhs=p[k,q]); v_aug column 64 is ones, so row 64 accumulates the softmax
  denominator. Normalization: reciprocal of that row, broadcast to 64
  partitions via a DRAM bounce DMA, multiplied into attnT.
  Out-proj: matmul(lhsT=attnT[c, s-block], rhs=wo[c, e-chunk]) -> [2048,1024]
  partial, DMA'd out.
All matmuls run as float32r (1 cycle/row at N>=256; fp32 data, reduced
internal precision).
"""

import os
import sys
import types

import numpy as np


def _ensure_axon_hooks_shim():
    """The agent image's antenv package lacks the axon_hooks submodule that
    concourse's trace path imports; install a stub so trace requests degrade
    to no-trace instead of crashing (a real hook can be set into the stub)."""
    try:
        import antenv.axon_hooks  # noqa: F401

        return
    except ImportError:
        pass
    try:
        import antenv
    except ImportError:
        return
    mod = types.ModuleType("antenv.axon_hooks")
    mod._AXON_NTFF_PROFILE_HOOK = None

    def get_axon_ntff_profile_hook():
        return mod._AXON_NTFF_PROFILE_HOOK

    def set_axon_ntff_profile_hook(hook):
        mod._AXON_NTFF_PROFILE_HOOK = hook

    mod.get_axon_ntff_profile_hook = get_axon_ntff_profile_hook
    mod.set_axon_ntff_profile_hook = set_axon_ntff_profile_hook
    sys.modules["antenv.axon_hooks"] = mod
    antenv.axon_hooks = mod


_ensure_axon_hooks_shim()

import concourse.bass as bass
import concourse.bacc as bacc
import concourse.mybir as mybir
import concourse.tile as tile
from concourse.bass_utils import run_bass_kernel_spmd

F32 = mybir.dt.float32
F32R = mybir.dt.float32r
F16 = mybir.dt.float16
AF = mybir.ActivationFunctionType
OP = mybir.AluOpType

B, DIM = 2, 1024
NH, NKV, HD = 16, 4, 64
GH = NH // NKV  # query heads per kv group = 4
S_FULL = 2048
SC = 512  # q chunk width
EXPG = 2  # score psum banks exp'd per ACT call


def build_nc(S=S_FULL, n_cores=8):
    NCH = S // SC
    NKB = S // 128
    KT = DIM // 128  # 8 k-tiles over the model dim

    nc = bacc.Bacc(
        "TRN2", target_bir_lowering=False, debug=False, num_devices=n_cores
    )
    xT = nc.dram_tensor("xT", [DIM, S], F16, kind="ExternalInput").ap()
    wq = nc.dram_tensor("wq", [DIM, 256], F16, kind="ExternalInput").ap()
    wkv = nc.dram_tensor("wkv", [DIM, 128], F16, kind="ExternalInput").ap()
    wo = nc.dram_tensor("wo", [256, DIM], F16, kind="ExternalInput").ap()
    cosr = nc.dram_tensor("cosr", [128, S], F32, kind="ExternalInput").ap()
    sinr = nc.dram_tensor("sinr", [128, S], F32, kind="ExternalInput").ap()
    tri = nc.dram_tensor("tri", [128, 128], F16, kind="ExternalInput").ap()
    ident = nc.dram_tensor("ident", [128, 128], F16, kind="ExternalInput").ap()
    sel = nc.dram_tensor("sel", [64, 128], F16, kind="ExternalInput").ap()
    ntri = nc.dram_tensor("ntri", [128, 896], F16, kind="ExternalInput").ap()
    out = nc.dram_tensor("out", [S, DIM], F32, kind="ExternalOutput").ap()

    xT3 = xT.rearrange("(k p) s -> k p s", p=128)
    wq3 = wq.rearrange("(k p) m -> k p m", p=128)
    wkv3 = wkv.rearrange("(k p) m -> k p m", p=128)
    wo3 = wo.rearrange("(t p) e -> t p e", p=128)

    with tile.TileContext(nc) as tc:
        with tc.tile_pool(name="const", bufs=1) as cp:
            COS = cp.tile([128, S], F32, tag="COS")
            SIN = cp.tile([128, S], F32, tag="SIN")
            WQ = cp.tile([128, KT, 256], F16, tag="WQ")
            WKV = cp.tile([128, KT, 128], F16, tag="WKV")
            WO = cp.tile([128, 2, DIM], F16, tag="WO")
            TRI = cp.tile([128, 128], F16, tag="TRI")
            IDENT = cp.tile([128, 128], F16, tag="IDENT")
            RE = cp.tile([128, S], F16, tag="RE")
            IM = cp.tile([128, S], F16, tag="IM")
            KA4 = cp.tile([128, S], F16, tag="KA4")  # becomes KAB: [KA;KB;KA;KB]
            KB4 = cp.tile([128, S], F16, tag="KB4")
            REIM0 = cp.tile([128, S], F16, tag="REIM0")
            REIM1 = cp.tile([128, S], F16, tag="REIM1")
            SEL = cp.tile([64, 128], F16, tag="SEL")
            NTRI = cp.tile([128, 896], F16, tag="NTRI")
            VT = cp.tile([64, S], F16, tag="VT")
            VAUG = cp.tile([128, NKB, 128], F16, tag="VAUG")
            AT0 = cp.tile([128, S], F16, tag="AT0")
            AT1 = cp.tile([128, S], F16, tag="AT1")

            nc.sync.dma_start(COS[:], cosr)
            nc.sync.dma_start(SIN[:], sinr)
            nc.sync.dma_start(TRI[:], tri)
            nc.sync.dma_start(IDENT[:], ident)
            nc.sync.dma_start(SEL[:], sel)
            nc.sync.dma_start(NTRI[:], ntri)
            nc.sync.dma_start(
                WQ[:], wq.rearrange("(k p) m -> p k m", p=128)
            )
            nc.sync.dma_start(
                WKV[:], wkv.rearrange("(k p) m -> p k m", p=128)
            )
            nc.sync.dma_start(
                WO[:], wo.rearrange("(t p) e -> p t e", p=128)
            )
            ONES = cp.tile([1, 64], F16, tag="ONES")
            nc.vector.memset(ONES[:], 1.0)
            nc.vector.memset(VAUG[:], 1.0)

            # ---- Phase A: projections + rope + v transpose ----
            with (
                tc.tile_pool(name="apsum", bufs=2, space="PSUM") as aps,
                tc.tile_pool(name="xt", bufs=3) as xp,
                tc.tile_pool(name="rt", bufs=3) as rt,
            ):
                for qc in range(NCH):
                    sl = slice(qc * SC, (qc + 1) * SC)
                    q0 = aps.tile([128, SC], F32, tag="q0")
                    q1 = aps.tile([128, SC], F32, tag="q1")
                    kv = aps.tile([128, SC], F32, tag="kv")
                    for kt in range(KT):
                        xt_t = xp.tile([128, SC], F16, tag="xt")
                        nc.sync.dma_start(xt_t[:], xT3[kt, :, sl])
                        st, sp = kt == 0, kt == KT - 1
                        nc.tensor.matmul(
                            q0[:], WQ[:, kt, 0:128],
                            xt_t[:], start=st, stop=sp,
                        )
                        nc.tensor.matmul(
                            q1[:], WQ[:, kt, 128:256],
                            xt_t[:], start=st, stop=sp,
                        )
                        nc.tensor.matmul(
                            kv[:], WKV[:, kt, :],
                            xt_t[:], start=st, stop=sp,
                        )
                    # rope q: RE = te*cos - to*sin ; IM = te*sin + to*cos
                    t1 = rt.tile([128, SC], F32, tag="t1")
                    t2 = rt.tile([128, SC], F32, tag="t2")
                    nc.vector.tensor_tensor(t1[:], q0[:], COS[:, sl], OP.mult)
                    nc.vector.tensor_tensor(t2[:], q1[:], SIN[:, sl], OP.mult)
                    nc.vector.tensor_tensor(RE[:, sl], t1[:], t2[:], OP.subtract)
                    t3 = rt.tile([128, SC], F32, tag="t3")
                    t4 = rt.tile([128, SC], F32, tag="t4")
                    nc.vector.tensor_tensor(t3[:], q0[:], SIN[:, sl], OP.mult)
                    nc.vector.tensor_tensor(t4[:], q1[:], COS[:, sl], OP.mult)
                    nc.vector.tensor_tensor(IM[:, sl], t3[:], t4[:], OP.add)
                    # rope k: kv rows 0:32 = evens, 32:64 = odds
                    u1 = rt.tile([32, SC], F32, tag="u1")
                    u2 = rt.tile([32, SC], F32, tag="u2")
                    nc.vector.tensor_tensor(u1[:], kv[0:32, :], COS[0:32, sl], OP.mult)
                    nc.vector.tensor_tensor(u2[:], kv[32:64, :], SIN[0:32, sl], OP.mult)
                    nc.vector.tensor_tensor(KA4[0:32, sl], u1[:], u2[:], OP.subtract)
                    u3 = rt.tile([32, SC], F32, tag="u3")
                    u4 = rt.tile([32, SC], F32, tag="u4")
                    nc.vector.tensor_tensor(u3[:], kv[0:32, :], SIN[0:32, sl], OP.mult)
                    nc.vector.tensor_tensor(u4[:], kv[32:64, :], COS[0:32, sl], OP.mult)
                    nc.vector.tensor_tensor(KB4[0:32, sl], u3[:], u4[:], OP.add)
                    # v
                    nc.vector.tensor_copy(VT[0:64, sl], kv[64:128, :])
                # build KAB = [KA;KB;KA;KB] in KA4, and the per-head-pair
                # interleaved [RE_h;IM_h] rhs tiles (SBUF->SBUF DMA)
                nc.gpsimd.dma_start(KA4[32:64, :], KB4[0:32, :])
                nc.gpsimd.dma_start(KA4[64:96, :], KA4[0:32, :])
                nc.gpsimd.dma_start(KA4[96:128, :], KB4[0:32, :])
                for t, RT in enumerate((REIM0, REIM1)):
                    for half in range(2):
                        h = 2 * t + half
                        rq = slice(32 * h, 32 * h + 32)
                        nc.gpsimd.dma_start(RT[64 * half : 64 * half + 32, :], RE[rq, :])
                        nc.gpsimd.dma_start(RT[64 * half + 32 : 64 * half + 64, :], IM[rq, :])
                # v transpose: vT [64, S] -> v_aug [k, 65] blocks
                for kb in range(NKB):
                    vp = aps.tile([128, 64], F16, tag="vp")
                    nc.tensor.transpose(
                        vp[:],
                        VT[0:64, kb * 128 : (kb + 1) * 128],
                        IDENT[0:64, 0:64],
                    )
                    nc.vector.tensor_copy(VAUG[:, kb, 0:HD], vp[:])

            # ---- Phase B: attention ----
            with (
                tc.tile_pool(name="bpsum", bufs=3, space="PSUM") as bps,
                tc.tile_pool(name="opsum", bufs=1, space="PSUM") as ops,
                tc.tile_pool(name="pp", bufs=4) as pp,
                tc.tile_pool(name="np_", bufs=2) as npo,
            ):
                for qc in range(NCH):
                    qsl = slice(qc * SC, (qc + 1) * SC)
                    nkb = 4 * qc + 4
                    for pr in range(2):  # head pairs (0,1) and (2,3)
                        RT = (REIM0, REIM1)[pr]
                        ot0 = ops.tile([128, SC], F32, tag="ot0")
                        ot1 = ops.tile([128, SC], F32, tag="ot1")
                        ots = (ot0, ot1)
                        def emit_pv(kb, p_sb):
                            for j in range(2):
                                nc.tensor.matmul(
                                    ots[j][:],
                                    VAUG[:, kb, :],
                                    p_sb[:, j, :],
                                    start=(kb == 0), stop=(kb == nkb - 1),
                                )

                        staged = []
                        for kb in range(nkb):
                            ksl = slice(kb * 128, (kb + 1) * 128)
                            jj = kb - 4 * qc
                            sc_ps = bps.tile([128, 2, SC], F32, tag="sc")
                            for j in range(2):  # head-in-pair; strips alternate
                                rs = slice(64 * j, 64 * j + 64)
                                nc.tensor.matmul(
                                    sc_ps[:, j, :],
                                    KA4[rs, ksl],
                                    RT[rs, qsl],
                                    start=True, stop=(jj < 0),
                                    tile_position=(64 * j, 0),
                                )
                            if jj >= 0:
                                # causal mask: accumulate a -1e4 upper strip
                                nsl = slice(384 - 128 * jj, 896 - 128 * jj)
                                for j in range(2):
                                    nc.tensor.matmul(
                                        sc_ps[:, j, :],
                                        IDENT[:],
                                        NTRI[:, nsl],
                                        start=False, stop=True,
                                    )
                            p_sb = pp.tile([128, 2, SC], F16, tag="p")
                            nc.scalar.activation(
                                p_sb[:], sc_ps[:], AF.Exp, scale=0.125
                            )
                            staged.append((kb, p_sb))
                            if len(staged) > 2:
                                emit_pv(*staged.pop(0))
                        for it in staged:
                            emit_pv(*it)
                                                # normalize both heads of the pair: denom rows ->
                        # one batched reciprocal -> SEL-matmul broadcast
                        den = npo.tile([64, SC], F32, tag="den")
                        nc.vector.memset(den[:], 1.0)
                        nc.vector.tensor_copy(den[0:1, :], ot0[64:65, :])
                        nc.vector.tensor_copy(den[32:33, :], ot1[64:65, :])
                        rec = npo.tile([64, SC], F16, tag="rec")
                        with nc.allow_low_precision(
                            reason="fp16 softmax denominators"
                        ):
                            nc.vector.reciprocal(rec[:], den[:])
                        rbc_ps = bps.tile([128, 2, SC], F32, tag="sc")
                        nc.tensor.matmul(
                            rbc_ps[:, 0, :], SEL[:], rec[:], start=True, stop=True
                        )
                        rbc_sb = npo.tile([128, SC], F32, tag="rbc_sb")
                        nc.vector.tensor_copy(rbc_sb[:], rbc_ps[:, 0, :])
                        att = (AT0, AT1)[pr]
                        nc.vector.tensor_tensor(
                            att[0:64, qsl], ot0[0:64, :], rbc_sb[0:64, :], OP.mult
                        )
                        nc.vector.tensor_tensor(
                            att[64:128, qsl], ot1[0:64, :], rbc_sb[64:128, :], OP.mult
                        )

            # ---- Phase C: output projection ----
            with (
                tc.tile_pool(name="cpsum", bufs=2, space="PSUM") as cps,
                tc.tile_pool(name="op", bufs=3) as op_pool,
            ):
                for sb_i in range(S // 128):
                    ssl = slice(sb_i * 128, (sb_i + 1) * 128)
                    for ec in range(DIM // 512):
                        esl = slice(ec * 512, (ec + 1) * 512)
                        o_ps = cps.tile([128, 512], F32, tag="o")
                        for t in range(2):
                            att = (AT0, AT1)[t]
                            nc.tensor.matmul(
                                o_ps[:],
                                att[:, ssl],
                                WO[:, t, esl],
                                start=(t == 0), stop=(t == 1),
                            )
                        ost = op_pool.tile([128, 512], F32, tag="ost")
                        nc.any.tensor_copy(ost[:], o_ps[:])
                        nc.scalar.dma_start(out[ssl, esl], ost[:])

    nc.compile()
    return nc


# host-side column permutations: all rope-even dims first, then all odds
_PERM256 = np.array(
    [64 * h + 2 * i for h in range(4) for i in range(32)]
    + [64 * h + 2 * i + 1 for h in range(4) for i in range(32)]
)
_PERM64 = np.array([2 * i for i in range(32)] + [2 * i + 1 for i in range(32)])

_cache = {}


def make_in_maps(x, cos, sin, wq, wk, wv, wo, n_groups=4):
    S = x.shape[1]
    cos_r = np.ascontiguousarray(np.tile(cos.T, (4, 1)), dtype=np.float32)
    sin_r = np.ascontiguousarray(np.tile(sin.T, (4, 1)), dtype=np.float32)
    tri = np.triu(np.ones((128, 128), dtype=np.float16))
    ident = np.eye(128, dtype=np.float16)
    sel = np.zeros((64, 128), dtype=np.float16)
    sel[0, 0:64] = 1.0
    sel[32, 64:128] = 1.0
    uu, pp_ = np.meshgrid(np.arange(896), np.arange(128))
    ntri = np.where(uu < pp_ + 384, np.float16(-10000.0), np.float16(0.0)).astype(
        np.float16
    )
    xTs = [np.ascontiguousarray(x[b].T.astype(np.float16)) for b in range(x.shape[0])]
    in_maps = []
    for c in range(x.shape[0] * n_groups):
        b, g = divmod(c, n_groups)
        wq_c = np.ascontiguousarray(wq[:, 256 * g + _PERM256].astype(np.float16))
        wk_c = wk[:, 64 * g + _PERM64]
        wv_c = wv[:, 64 * g : 64 * (g + 1)]
        wkv_c = np.ascontiguousarray(
            np.concatenate([wk_c, wv_c], axis=1), dtype=np.float16
        )
        wo_c = np.ascontiguousarray(wo[256 * g : 256 * (g + 1), :].astype(np.float16))
        in_maps.append(
            {
                "xT": xTs[b],
                "wq": wq_c,
                "wkv": wkv_c,
                "wo": wo_c,
                "cosr": cos_r,
                "sinr": sin_r,
                "tri": tri,
                "ident": ident,
                "sel": sel,
                "ntri": ntri,
            }
        )
    return in_maps


def kernel(x, cos, sin, mask, wq, wk, wv, wo):
    x = np.asarray(x, dtype=np.float32)
    cos = np.asarray(cos, dtype=np.float32)
    sin = np.asarray(sin, dtype=np.float32)
    wq = np.asarray(wq, dtype=np.float32)
    wk = np.asarray(wk, dtype=np.float32)
    wv = np.asarray(wv, dtype=np.float32)
    wo = np.asarray(wo, dtype=np.float32)

    if "nc" not in _cache:
        _cache["nc"] = build_nc(S=x.shape[1], n_cores=8)
    nc = _cache["nc"]
    in_maps = make_in_maps(x, cos, sin, wq, wk, wv, wo)
    res = run_bass_kernel_spmd(nc, in_maps, list(range(8)))
    _cache["last"] = res
    outs = [r["out"] for r in res.results]
    final = np.stack(
        [outs[0] + outs[1] + outs[2] + outs[3], outs[4] + outs[5] + outs[6] + outs[7]],
        axis=0,
    )
    return final.astype(np.float32)


# revision 15
# speedup vs baseline: 1.1123x; 1.0935x over previous
"""GQA attention (RoPE + causal softmax + out-proj) on 8 TRN2 cores.

Sharding: one core per (batch b, kv-head-group g): 2 batches x 4 kv groups = 8
cores. Each core computes its group's 4 query heads end to end, including the
partial output projection through its 256 rows of wo; the host sums the 4
partial projections per batch (the wo row-shard all-reduce is done on host).

Per-core kernel layout (all "transposed domain": feature dims on partitions,
sequence on the free axis):
  qT [256, S] = wq_g^T x^T, computed as matmul(lhsT=wq_g, rhs=xT); wq columns
  are host-permuted so psum M-tile 0 holds all even (te) rope components
  (4 heads x 32) and M-tile 1 all odd (to). RoPE is then whole-tile vector
  ops against cos/sin tables replicated per head. Same for k (rows 0:64 of
  the packed kv projection; v = rows 64:128).
  scores_T [k 128, q 512] per (head, k-block, q-chunk) = two K=32 row-packed
  matmuls (evens + odds strips) accumulated in psum; exp via ACT (scale=1/8,
  no max subtraction -- scores are O(1) by construction); causality via
  memset + triangular mask multiply on the diagonal blocks only (blocks
  above the diagonal are never computed).
  PV: out_T [65, q 512] accumulated over k-blocks: matmul(lhsT=v_aug[k,65],
  rhs=p[k,q]); v_aug column 64 is ones, so row 64 accumulates the softmax
  denominator. Normalization: reciprocal of that row, broadcast to 64
  partitions via a DRAM bounce DMA, multiplied into attnT.
  Out-proj: matmul(lhsT=attnT[c, s-block], rhs=wo[c, e-chunk]) -> [2048,1024]
  partial, DMA'd out.
All matmuls run as float32r (1 cycle/row at N>=256; fp32 data, reduced
internal precision).
"""

import os
import sys
import types

import numpy as np


def _ensure_axon_hooks_shim():
    """The agent image's antenv package lacks the axon_hooks submodule that
    concourse's trace path imports; install a stub so trace requests degrade
    to no-trace instead of crashing (a real hook can be set into the stub)."""
    try:
        import antenv.axon_hooks  # noqa: F401

        return
    except ImportError:
        pass
    try:
        import antenv
    except ImportError:
        return
    mod = types.ModuleType("antenv.axon_hooks")
    mod._AXON_NTFF_PROFILE_HOOK = None

    def get_axon_ntff_profile_hook():
        return mod._AXON_NTFF_PROFILE_HOOK

    def set_axon_ntff_profile_hook(hook):
        mod._AXON_NTFF_PROFILE_HOOK = hook

    mod.get_axon_ntff_profile_hook = get_axon_ntff_profile_hook
    mod.set_axon_ntff_profile_hook = set_axon_ntff_profile_hook
    sys.modules["antenv.axon_hooks"] = mod
    antenv.axon_hooks = mod


_ensure_axon_hooks_shim()

import concourse.bass as bass
import concourse.bacc as bacc
import concourse.mybir as mybir
import concourse.tile as tile
from concourse.bass_utils import run_bass_kernel_spmd

F32 = mybir.dt.float32
F32R = mybir.dt.float32r
F16 = mybir.dt.float16
AF = mybir.ActivationFunctionType
OP = mybir.AluOpType

B, DIM = 2, 1024
NH, NKV, HD = 16, 4, 64
GH = NH // NKV  # query heads per kv group = 4
S_FULL = 2048
SC = 512  # q chunk width
EXPG = 2  # score psum banks exp'd per ACT call


def build_nc(S=S_FULL, n_cores=8):
    NCH = S // SC
    NKB = S // 128
    KT = DIM // 128  # 8 k-tiles over the model dim

    nc = bacc.Bacc(
        "TRN2", target_bir_lowering=False, debug=False, num_devices=n_cores
    )
    xT = nc.dram_tensor("xT", [DIM, S], F16, kind="ExternalInput").ap()
    wq = nc.dram_tensor("wq", [DIM, 256], F16, kind="ExternalInput").ap()
    wkv = nc.dram_tensor("wkv", [DIM, 128], F16, kind="ExternalInput").ap()
    wo = nc.dram_tensor("wo", [256, DIM], F16, kind="ExternalInput").ap()
    cosr = nc.dram_tensor("cosr", [128, S], F32, kind="ExternalInput").ap()
    sinr = nc.dram_tensor("sinr", [128, S], F32, kind="ExternalInput").ap()
    tri = nc.dram_tensor("tri", [128, 128], F16, kind="ExternalInput").ap()
    ident = nc.dram_tensor("ident", [128, 128], F16, kind="ExternalInput").ap()
    sel = nc.dram_tensor("sel", [64, 128], F16, kind="ExternalInput").ap()
    ntri = nc.dram_tensor("ntri", [128, 896], F16, kind="ExternalInput").ap()
    out = nc.dram_tensor("out", [S, DIM], F32, kind="ExternalOutput").ap()

    xT3 = xT.rearrange("(k p) s -> k p s", p=128)

    with tile.TileContext(nc) as tc:
        with (
            tc.tile_pool(name="const", bufs=1) as cp,
            tc.tile_pool(name="qps", bufs=2, space="PSUM") as qps,
            tc.tile_pool(name="scps", bufs=2, space="PSUM") as bps,
            tc.tile_pool(name="otps", bufs=2, space="PSUM") as ops,
            tc.tile_pool(name="xt", bufs=16) as xp,
            tc.tile_pool(name="rt", bufs=3) as rt,
            tc.tile_pool(name="pp", bufs=4) as pp,
            tc.tile_pool(name="np_", bufs=2) as npo,
            tc.tile_pool(name="op", bufs=3) as op_pool,
        ):
            COS = cp.tile([128, S], F32, tag="COS")
            SIN = cp.tile([128, S], F32, tag="SIN")
            WQ = cp.tile([128, KT, 256], F16, tag="WQ")
            WKV = cp.tile([128, KT, 128], F16, tag="WKV")
            WO = cp.tile([128, 2, DIM], F16, tag="WO")
            TRI = cp.tile([128, 128], F16, tag="TRI")
            IDENT = cp.tile([128, 128], F16, tag="IDENT")
            SEL = cp.tile([64, 128], F16, tag="SEL")
            NTRI = cp.tile([128, 896], F16, tag="NTRI")
            RE = cp.tile([128, S], F16, tag="RE")
            IM = cp.tile([128, S], F16, tag="IM")
            KA4 = cp.tile([128, S], F16, tag="KA4")  # KAB: [KA;KB;KA;KB]
            KB4 = cp.tile([32, S], F16, tag="KB4")
            REIM0 = cp.tile([128, S], F16, tag="REIM0")
            REIM1 = cp.tile([128, S], F16, tag="REIM1")
            VT = cp.tile([64, S], F16, tag="VT")
            VAUG = cp.tile([128, NKB, 128], F16, tag="VAUG")
            AT0 = cp.tile([128, S], F16, tag="AT0")
            AT1 = cp.tile([128, S], F16, tag="AT1")

            nc.sync.dma_start(COS[:], cosr)
            nc.sync.dma_start(SIN[:], sinr)
            nc.sync.dma_start(TRI[:], tri)
            nc.sync.dma_start(IDENT[:], ident)
            nc.sync.dma_start(SEL[:], sel)
            nc.sync.dma_start(NTRI[:], ntri)
            nc.sync.dma_start(WQ[:], wq.rearrange("(k p) m -> p k m", p=128))
            nc.sync.dma_start(WKV[:], wkv.rearrange("(k p) m -> p k m", p=128))
            nc.sync.dma_start(WO[:], wo.rearrange("(t p) e -> p t e", p=128))
            nc.vector.memset(VAUG[:], 1.0)

            for qc in range(NCH):
                sl = slice(qc * SC, (qc + 1) * SC)
                nkb = 4 * qc + 4
                qsl = sl

                # ---- projections for this chunk (two passes over 2 shared
                # psum slots so attention's 6 banks stay free) ----
                q0 = qps.tile([128, SC], F32, tag="q")
                q1 = qps.tile([128, SC], F32, tag="q")
                xts = []
                for kt in range(KT):
                    xt_t = xp.tile([128, SC], F16, tag="xt")
                    nc.sync.dma_start(xt_t[:], xT3[kt, :, sl])
                    xts.append(xt_t)
                    st, sp = kt == 0, kt == KT - 1
                    nc.tensor.matmul(
                        q0[:], WQ[:, kt, 0:128], xt_t[:], start=st, stop=sp
                    )
                    nc.tensor.matmul(
                        q1[:], WQ[:, kt, 128:256], xt_t[:], start=st, stop=sp
                    )
                # rope q -> RE/IM, then scatter into the per-pair REIM tiles
                t1 = rt.tile([128, SC], F32, tag="t1")
                t2 = rt.tile([128, SC], F32, tag="t2")
                nc.vector.tensor_tensor(t1[:], q0[:], COS[:, sl], OP.mult)
                nc.vector.tensor_tensor(t2[:], q1[:], SIN[:, sl], OP.mult)
                nc.vector.tensor_tensor(RE[:, sl], t1[:], t2[:], OP.subtract)
                t3 = rt.tile([128, SC], F32, tag="t3")
                t4 = rt.tile([128, SC], F32, tag="t4")
                nc.vector.tensor_tensor(t3[:], q0[:], SIN[:, sl], OP.mult)
                nc.vector.tensor_tensor(t4[:], q1[:], COS[:, sl], OP.mult)
                nc.vector.tensor_tensor(IM[:, sl], t3[:], t4[:], OP.add)
                for t, RT_ in enumerate((REIM0, REIM1)):
                    for half in range(2):
                        h = 2 * t + half
                        rq = slice(32 * h, 32 * h + 32)
                        nc.sync.dma_start(
                            RT_[64 * half : 64 * half + 32, sl], RE[rq, sl]
                        )
                        nc.sync.dma_start(
                            RT_[64 * half + 32 : 64 * half + 64, sl], IM[rq, sl]
                        )
                # kv pass (reuses the resident xt tiles)
                kv = qps.tile([128, SC], F32, tag="q")
                for kt in range(KT):
                    nc.tensor.matmul(
                        kv[:], WKV[:, kt, :], xts[kt][:],
                        start=(kt == 0), stop=(kt == KT - 1),
                    )
                u1 = rt.tile([32, SC], F32, tag="u1")
                u2 = rt.tile([32, SC], F32, tag="u2")
                nc.vector.tensor_tensor(u1[:], kv[0:32, :], COS[0:32, sl], OP.mult)
                nc.vector.tensor_tensor(u2[:], kv[32:64, :], SIN[0:32, sl], OP.mult)
                nc.vector.tensor_tensor(KA4[0:32, sl], u1[:], u2[:], OP.subtract)
                u3 = rt.tile([32, SC], F32, tag="u3")
                u4 = rt.tile([32, SC], F32, tag="u4")
                nc.vector.tensor_tensor(u3[:], kv[0:32, :], SIN[0:32, sl], OP.mult)
                nc.vector.tensor_tensor(u4[:], kv[32:64, :], COS[0:32, sl], OP.mult)
                nc.vector.tensor_tensor(KB4[0:32, sl], u3[:], u4[:], OP.add)
                nc.sync.dma_start(KA4[32:64, sl], KB4[0:32, sl])
                nc.sync.dma_start(KA4[64:96, sl], KA4[0:32, sl])
                nc.sync.dma_start(KA4[96:128, sl], KB4[0:32, sl])
                nc.vector.tensor_copy(VT[0:64, sl], kv[64:128, :])
                for kb in range(4 * qc, 4 * qc + 4):
                    vp = ops.tile([128, 64], F16, tag="ot")
                    nc.tensor.transpose(
                        vp[:], VT[0:64, kb * 128 : (kb + 1) * 128],
                        IDENT[0:64, 0:64],
                    )
                    nc.vector.tensor_copy(VAUG[:, kb, 0:HD], vp[:])

                # ---- attention for this chunk ----
                for pr in range(2):  # head pairs (0,1) and (2,3)
                    RT_ = (REIM0, REIM1)[pr]
                    ot0 = ops.tile([128, SC], F32, tag="ot")
                    ot1 = ops.tile([128, SC], F32, tag="ot")
                    ots = (ot0, ot1)

                    def emit_pv(kb, p_sb):
                        for j in range(2):
                            nc.tensor.matmul(
                                ots[j][:], VAUG[:, kb, :], p_sb[:, j, :],
                                start=(kb == 0), stop=(kb == nkb - 1),
                            )

                    staged = []
                    for kb in range(nkb):
                        ksl = slice(kb * 128, (kb + 1) * 128)
                        jj = kb - 4 * qc
                        sc_ps = bps.tile([128, 2, SC], F32, tag="sc")
                        for j in range(2):
                            rs = slice(64 * j, 64 * j + 64)
                            nc.tensor.matmul(
                                sc_ps[:, j, :], KA4[rs, ksl], RT_[rs, qsl],
                                start=True, stop=(jj < 0),
                                tile_position=(64 * j, 0),
                            )
                        if jj >= 0:
                            nsl = slice(384 - 128 * jj, 896 - 128 * jj)
                            for j in range(2):
                                nc.tensor.matmul(
                                    sc_ps[:, j, :], IDENT[:], NTRI[:, nsl],
                                    start=False, stop=True,
                                )
                        p_sb = pp.tile([128, 2, SC], F16, tag="p")
                        nc.scalar.activation(
                            p_sb[:], sc_ps[:], AF.Exp, scale=0.125
                        )
                        staged.append((kb, p_sb))
                        if len(staged) > 2:
                            emit_pv(*staged.pop(0))
                    for it in staged:
                        emit_pv(*it)

                    den = npo.tile([64, SC], F32, tag="den")
                    nc.vector.memset(den[:], 1.0)
                    nc.vector.tensor_copy(den[0:1, :], ot0[64:65, :])
                    nc.vector.tensor_copy(den[32:33, :], ot1[64:65, :])
                    rec = npo.tile([64, SC], F16, tag="rec")
                    with nc.allow_low_precision(
                        reason="fp16 softmax denominators"
                    ):
                        nc.vector.reciprocal(rec[:], den[:])
                    rbc_ps = bps.tile([128, 2, SC], F32, tag="sc")
                    nc.tensor.matmul(
                        rbc_ps[:, 0, :], SEL[:], rec[:], start=True, stop=True
                    )
                    rbc_sb = npo.tile([128, SC], F32, tag="rbc_sb")
                    nc.vector.tensor_copy(rbc_sb[:], rbc_ps[:, 0, :])
                    att = (AT0, AT1)[pr]
                    nc.vector.tensor_tensor(
                        att[0:64, qsl], ot0[0:64, :], rbc_sb[0:64, :], OP.mult
                    )
                    nc.vector.tensor_tensor(
                        att[64:128, qsl], ot1[0:64, :], rbc_sb[64:128, :], OP.mult
                    )

                # ---- output projection for this chunk's s-blocks ----
                for sb_i in range(4 * qc, 4 * qc + 4):
                    ssl = slice(sb_i * 128, (sb_i + 1) * 128)
                    for ec in range(DIM // 512):
                        esl = slice(ec * 512, (ec + 1) * 512)
                        o_ps = bps.tile([128, 2, SC], F32, tag="sc")
                        for t in range(2):
                            att = (AT0, AT1)[t]
                            nc.tensor.matmul(
                                o_ps[:, 0, :], att[:, ssl], WO[:, t, esl],
                                start=(t == 0), stop=(t == 1),
                            )
                        ost = op_pool.tile([128, 512], F32, tag="ost")
                        nc.any.tensor_copy(ost[:], o_ps[:, 0, :])
                        nc.scalar.dma_start(out[ssl, esl], ost[:])

    nc.compile()
    return nc


# host-side column permutations: all rope-even dims first, then all odds
_PERM256 = np.array(
    [64 * h + 2 * i for h in range(4) for i in range(32)]
    + [64 * h + 2 * i + 1 for h in range(4) for i in range(32)]
)
_PERM64 = np.array([2 * i for i in range(32)] + [2 * i + 1 for i in range(32)])

_cache = {}


def make_in_maps(x, cos, sin, wq, wk, wv, wo, n_groups=4):
    S = x.shape[1]
    cos_r = np.ascontiguousarray(np.tile(cos.T, (4, 1)), dtype=np.float32)
    sin_r = np.ascontiguousarray(np.tile(sin.T, (4, 1)), dtype=np.float32)
    tri = np.triu(np.ones((128, 128), dtype=np.float16))
    ident = np.eye(128, dtype=np.float16)
    sel = np.zeros((64, 128), dtype=np.float16)
    sel[0, 0:64] = 1.0
    sel[32, 64:128] = 1.0
    uu, pp_ = np.meshgrid(np.arange(896), np.arange(128))
    ntri = np.where(uu < pp_ + 384, np.float16(-10000.0), np.float16(0.0)).astype(
        np.float16
    )
    xTs = [np.ascontiguousarray(x[b].T.astype(np.float16)) for b in range(x.shape[0])]
    in_maps = []
    for c in range(x.shape[0] * n_groups):
        b, g = divmod(c, n_groups)
        wq_c = np.ascontiguousarray(wq[:, 256 * g + _PERM256].astype(np.float16))
        wk_c = wk[:, 64 * g + _PERM64]
        wv_c = wv[:, 64 * g : 64 * (g + 1)]
        wkv_c = np.ascontiguousarray(
            np.concatenate([wk_c, wv_c], axis=1), dtype=np.float16
        )
        wo_c = np.ascontiguousarray(wo[256 * g : 256 * (g + 1), :].astype(np.float16))
        in_maps.append(
            {
                "xT": xTs[b],
                "wq": wq_c,
                "wkv": wkv_c,
                "wo": wo_c,
                "cosr": cos_r,
                "sinr": sin_r,
                "tri": tri,
                "ident": ident,
                "sel": sel,
                "ntri": ntri,
            }
        )
    return in_maps


def kernel(x, cos, sin, mask, wq, wk, wv, wo):
    x = np.asarray(x, dtype=np.float32)
    cos = np.asarray(cos, dtype=np.float32)
    sin = np.asarray(sin, dtype=np.float32)
    wq = np.asarray(wq, dtype=np.float32)
    wk = np.asarray(wk, dtype=np.float32)
    wv = np.asarray(wv, dtype=np.float32)
    wo = np.asarray(wo, dtype=np.float32)

    if "nc" not in _cache:
        _cache["nc"] = build_nc(S=x.shape[1], n_cores=8)
    nc = _cache["nc"]
    in_maps = make_in_maps(x, cos, sin, wq, wk, wv, wo)
    res = run_bass_kernel_spmd(nc, in_maps, list(range(8)))
    _cache["last"] = res
    outs = [r["out"] for r in res.results]
    final = np.stack(
        [outs[0] + outs[1] + outs[2] + outs[3], outs[4] + outs[5] + outs[6] + outs[7]],
        axis=0,
    )
    return final.astype(np.float32)


# revision 17
# speedup vs baseline: 1.1299x; 1.0158x over previous
"""GQA attention (RoPE + causal softmax + out-proj) on 8 TRN2 cores.

Sharding: one core per (batch b, kv-head-group g): 2 batches x 4 kv groups = 8
cores. Each core computes its group's 4 query heads end to end, including the
partial output projection through its 256 rows of wo; the host sums the 4
partial projections per batch (the wo row-shard all-reduce is done on host).

Per-core kernel layout (all "transposed domain": feature dims on partitions,
sequence on the free axis):
  qT [256, S] = wq_g^T x^T, computed as matmul(lhsT=wq_g, rhs=xT); wq columns
  are host-permuted so psum M-tile 0 holds all even (te) rope components
  (4 heads x 32) and M-tile 1 all odd (to). RoPE is then whole-tile vector
  ops against cos/sin tables replicated per head. Same for k (rows 0:64 of
  the packed kv projection; v = rows 64:128).
  scores_T [k 128, q 512] per (head, k-block, q-chunk) = two K=32 row-packed
  matmuls (evens + odds strips) accumulated in psum; exp via ACT (scale=1/8,
  no max subtraction -- scores are O(1) by construction); causality via
  memset + triangular mask multiply on the diagonal blocks only (blocks
  above the diagonal are never computed).
  PV: out_T [65, q 512] accumulated over k-blocks: matmul(lhsT=v_aug[k,65],
  rhs=p[k,q]); v_aug column 64 is ones, so row 64 accumulates the softmax
  denominator. Normalization: reciprocal of that row, broadcast to 64
  partitions via a DRAM bounce DMA, multiplied into attnT.
  Out-proj: matmul(lhsT=attnT[c, s-block], rhs=wo[c, e-chunk]) -> [2048,1024]
  partial, DMA'd out.
All matmuls run as float32r (1 cycle/row at N>=256; fp32 data, reduced
internal precision).
"""

import os
import sys
import types

import numpy as np


def _ensure_axon_hooks_shim():
    """The agent image's antenv package lacks the axon_hooks submodule that
    concourse's trace path imports; install a stub so trace requests degrade
    to no-trace instead of crashing (a real hook can be set into the stub)."""
    try:
        import antenv.axon_hooks  # noqa: F401

        return
    except ImportError:
        pass
    try:
        import antenv
    except ImportError:
        return
    mod = types.ModuleType("antenv.axon_hooks")
    mod._AXON_NTFF_PROFILE_HOOK = None

    def get_axon_ntff_profile_hook():
        return mod._AXON_NTFF_PROFILE_HOOK

    def set_axon_ntff_profile_hook(hook):
        mod._AXON_NTFF_PROFILE_HOOK = hook

    mod.get_axon_ntff_profile_hook = get_axon_ntff_profile_hook
    mod.set_axon_ntff_profile_hook = set_axon_ntff_profile_hook
    sys.modules["antenv.axon_hooks"] = mod
    antenv.axon_hooks = mod


_ensure_axon_hooks_shim()

import concourse.bass as bass
import concourse.bacc as bacc
import concourse.mybir as mybir
import concourse.tile as tile
from concourse.bass_utils import run_bass_kernel_spmd

F32 = mybir.dt.float32
F32R = mybir.dt.float32r
F16 = mybir.dt.float16
AF = mybir.ActivationFunctionType
OP = mybir.AluOpType

B, DIM = 2, 1024
NH, NKV, HD = 16, 4, 64
GH = NH // NKV  # query heads per kv group = 4
S_FULL = 2048
SC = 512  # q chunk width
EXPG = 2  # score psum banks exp'd per ACT call


def build_nc(S=S_FULL, n_cores=8):
    NCH = S // SC
    NKB = S // 128
    KT = DIM // 128  # 8 k-tiles over the model dim

    nc = bacc.Bacc(
        "TRN2", target_bir_lowering=False, debug=False, num_devices=n_cores
    )
    xT = nc.dram_tensor("xT", [DIM, S], F16, kind="ExternalInput").ap()
    wq = nc.dram_tensor("wq", [DIM, 256], F16, kind="ExternalInput").ap()
    wkv = nc.dram_tensor("wkv", [DIM, 128], F16, kind="ExternalInput").ap()
    wo = nc.dram_tensor("wo", [256, DIM], F16, kind="ExternalInput").ap()
    cosr = nc.dram_tensor("cosr", [128, S], F16, kind="ExternalInput").ap()
    sinr = nc.dram_tensor("sinr", [128, S], F16, kind="ExternalInput").ap()
    tri = nc.dram_tensor("tri", [128, 128], F16, kind="ExternalInput").ap()
    ident = nc.dram_tensor("ident", [128, 128], F16, kind="ExternalInput").ap()
    sel = nc.dram_tensor("sel", [64, 128], F16, kind="ExternalInput").ap()
    ntri = nc.dram_tensor("ntri", [128, 896], F16, kind="ExternalInput").ap()
    out = nc.dram_tensor("out", [S, DIM], F32, kind="ExternalOutput").ap()

    xT3 = xT.rearrange("(k p) s -> k p s", p=128)

    with tile.TileContext(nc) as tc:
        with (
            tc.tile_pool(name="const", bufs=1) as cp,
            tc.tile_pool(name="qps", bufs=2, space="PSUM") as qps,
            tc.tile_pool(name="scps", bufs=2, space="PSUM") as bps,
            tc.tile_pool(name="otps", bufs=2, space="PSUM") as ops,
            tc.tile_pool(name="xt", bufs=16) as xp,
            tc.tile_pool(name="rt", bufs=3) as rt,
            tc.tile_pool(name="pp", bufs=4) as pp,
            tc.tile_pool(name="np_", bufs=2) as npo,
            tc.tile_pool(name="op", bufs=3) as op_pool,
        ):
            COS = cp.tile([128, S], F16, tag="COS")
            SIN = cp.tile([128, S], F16, tag="SIN")
            WQ = cp.tile([128, KT, 256], F16, tag="WQ")
            WKV = cp.tile([128, KT, 128], F16, tag="WKV")
            WO = cp.tile([128, 2, DIM], F16, tag="WO")
            TRI = cp.tile([128, 128], F16, tag="TRI")
            IDENT = cp.tile([128, 128], F16, tag="IDENT")
            SEL = cp.tile([64, 128], F16, tag="SEL")
            NTRI = cp.tile([128, 896], F16, tag="NTRI")
            KA4 = cp.tile([128, S], F16, tag="KA4")  # KAB: [KA;KB;KA;KB]
            REIM0 = cp.tile([128, S], F16, tag="REIM0")
            REIM1 = cp.tile([128, S], F16, tag="REIM1")
            VT = cp.tile([64, S], F16, tag="VT")
            VAUG = cp.tile([128, NKB, 128], F16, tag="VAUG")
            AT0 = cp.tile([128, S], F16, tag="AT0")
            AT1 = cp.tile([128, S], F16, tag="AT1")

            nc.sync.dma_start(COS[:], cosr)
            nc.sync.dma_start(SIN[:], sinr)
            nc.sync.dma_start(TRI[:], tri)
            nc.sync.dma_start(IDENT[:], ident)
            nc.sync.dma_start(SEL[:], sel)
            nc.sync.dma_start(NTRI[:], ntri)
            nc.sync.dma_start(WQ[:], wq.rearrange("(k p) m -> p k m", p=128))
            nc.sync.dma_start(WKV[:], wkv.rearrange("(k p) m -> p k m", p=128))
            nc.sync.dma_start(WO[:], wo.rearrange("(t p) e -> p t e", p=128))
            nc.vector.memset(VAUG[:], 1.0)

            for qc in range(NCH):
                sl = slice(qc * SC, (qc + 1) * SC)
                nkb = 4 * qc + 4
                qsl = sl

                # ---- projections for this chunk (two passes over 2 shared
                # psum slots so attention's 6 banks stay free) ----
                q0 = qps.tile([128, SC], F32, tag="q")
                q1 = qps.tile([128, SC], F32, tag="q")
                xts = []
                for kt in range(KT):
                    xt_t = xp.tile([128, SC], F16, tag="xt")
                    nc.sync.dma_start(xt_t[:], xT3[kt, :, sl])
                    xts.append(xt_t)
                    st, sp = kt == 0, kt == KT - 1
                    nc.tensor.matmul(
                        q0[:], WQ[:, kt, 0:128], xt_t[:], start=st, stop=sp
                    )
                    nc.tensor.matmul(
                        q1[:], WQ[:, kt, 128:256], xt_t[:], start=st, stop=sp
                    )
                # rope q: fp16 copies via ACT, 2x-mode DVE ops writing the
                # per-pair interleaved REIM tiles directly
                q0s = rt.tile([128, SC], F16, tag="q0s")
                q1s = rt.tile([128, SC], F16, tag="q1s")
                nc.scalar.copy(q0s[:], q0[:])
                nc.scalar.copy(q1s[:], q1[:])
                t1 = rt.tile([128, SC], F16, tag="t1")
                t2 = rt.tile([128, SC], F16, tag="t2")
                t3 = rt.tile([128, SC], F16, tag="t3")
                t4 = rt.tile([128, SC], F16, tag="t4")
                nc.vector.tensor_tensor(t1[:], q0s[:], COS[:, sl], OP.mult)
                nc.vector.tensor_tensor(t2[:], q1s[:], SIN[:, sl], OP.mult)
                nc.vector.tensor_tensor(t3[:], q0s[:], SIN[:, sl], OP.mult)
                nc.vector.tensor_tensor(t4[:], q1s[:], COS[:, sl], OP.mult)
                for t, RT_ in enumerate((REIM0, REIM1)):
                    for half in range(2):
                        h = 2 * t + half
                        rq = slice(32 * h, 32 * h + 32)
                        nc.vector.tensor_tensor(
                            RT_[64 * half : 64 * half + 32, sl],
                            t1[rq, :], t2[rq, :], OP.subtract,
                        )
                        nc.vector.tensor_tensor(
                            RT_[64 * half + 32 : 64 * half + 64, sl],
                            t3[rq, :], t4[rq, :], OP.add,
                        )
                # kv pass (reuses the resident xt tiles)
                kv = qps.tile([128, SC], F32, tag="q")
                for kt in range(KT):
                    nc.tensor.matmul(
                        kv[:], WKV[:, kt, :], xts[kt][:],
                        start=(kt == 0), stop=(kt == KT - 1),
                    )
                kvs = rt.tile([128, SC], F16, tag="kvs")
                nc.scalar.copy(kvs[:], kv[:])
                u1 = rt.tile([32, SC], F16, tag="u1")
                u2 = rt.tile([32, SC], F16, tag="u2")
                nc.vector.tensor_tensor(u1[:], kvs[0:32, :], COS[0:32, sl], OP.mult)
                nc.vector.tensor_tensor(u2[:], kvs[32:64, :], SIN[32:64, sl], OP.mult)
                nc.vector.tensor_tensor(KA4[0:32, sl], u1[:], u2[:], OP.subtract)
                u3 = rt.tile([32, SC], F16, tag="u3")
                u4 = rt.tile([32, SC], F16, tag="u4")
                nc.vector.tensor_tensor(u3[:], kvs[0:32, :], SIN[0:32, sl], OP.mult)
                nc.vector.tensor_tensor(u4[:], kvs[32:64, :], COS[32:64, sl], OP.mult)
                nc.vector.tensor_tensor(KA4[32:64, sl], u3[:], u4[:], OP.add)
                nc.sync.dma_start(KA4[64:128, sl], KA4[0:64, sl])
                nc.vector.tensor_copy(VT[0:64, sl], kvs[64:128, :])
                for kb in range(4 * qc, 4 * qc + 4):
                    vp = ops.tile([128, 64], F16, tag="ot")
                    nc.tensor.transpose(
                        vp[:], VT[0:64, kb * 128 : (kb + 1) * 128],
                        IDENT[0:64, 0:64],
                    )
                    nc.vector.tensor_copy(VAUG[:, kb, 0:HD], vp[:])

                # ---- attention for this chunk ----
                for pr in range(2):  # head pairs (0,1) and (2,3)
                    RT_ = (REIM0, REIM1)[pr]
                    ot0 = ops.tile([128, SC], F32, tag="ot")
                    ot1 = ops.tile([128, SC], F32, tag="ot")
                    ots = (ot0, ot1)

                    def emit_pv(kb, p_sb):
                        for j in range(2):
                            nc.tensor.matmul(
                                ots[j][:], VAUG[:, kb, :], p_sb[:, j, :],
                                start=(kb == 0), stop=(kb == nkb - 1),
                            )

                    staged = []
                    for kb in range(nkb):
                        ksl = slice(kb * 128, (kb + 1) * 128)
                        jj = kb - 4 * qc
                        sc_ps = bps.tile([128, 2, SC], F32, tag="sc")
                        for j in range(2):
                            rs = slice(64 * j, 64 * j + 64)
                            nc.tensor.matmul(
                                sc_ps[:, j, :], KA4[rs, ksl], RT_[rs, qsl],
                                start=True, stop=(jj < 0),
                                tile_position=(64 * j, 0),
                            )
                        if jj >= 0:
                            nsl = slice(384 - 128 * jj, 896 - 128 * jj)
                            for j in range(2):
                                nc.tensor.matmul(
                                    sc_ps[:, j, :], IDENT[:], NTRI[:, nsl],
                                    start=False, stop=True,
                                )
                        p_sb = pp.tile([128, 2, SC], F16, tag="p")
                        nc.scalar.activation(
                            p_sb[:], sc_ps[:], AF.Exp, scale=0.125
                        )
                        staged.append((kb, p_sb))
                        if len(staged) > 2:
                            emit_pv(*staged.pop(0))
                    for it in staged:
                        emit_pv(*it)

                    den = npo.tile([64, SC], F32, tag="den")
                    nc.vector.memset(den[:], 1.0)
                    nc.vector.tensor_copy(den[0:1, :], ot0[64:65, :])
                    nc.vector.tensor_copy(den[32:33, :], ot1[64:65, :])
                    rec = npo.tile([64, SC], F16, tag="rec")
                    with nc.allow_low_precision(
                        reason="fp16 softmax denominators"
                    ):
                        nc.vector.reciprocal(rec[:], den[:])
                    rbc_ps = bps.tile([128, 2, SC], F32, tag="sc")
                    nc.tensor.matmul(
                        rbc_ps[:, 0, :], SEL[:], rec[:], start=True, stop=True
                    )
                    rbc_sb = npo.tile([128, SC], F32, tag="rbc_sb")
                    nc.vector.tensor_copy(rbc_sb[:], rbc_ps[:, 0, :])
                    att = (AT0, AT1)[pr]
                    nc.vector.tensor_tensor(
                        att[0:64, qsl], ot0[0:64, :], rbc_sb[0:64, :], OP.mult
                    )
                    nc.vector.tensor_tensor(
                        att[64:128, qsl], ot1[0:64, :], rbc_sb[64:128, :], OP.mult
                    )

                # ---- output projection for this chunk's s-blocks ----
                for sb_i in range(4 * qc, 4 * qc + 4):
                    ssl = slice(sb_i * 128, (sb_i + 1) * 128)
                    for ec in range(DIM // 512):
                        esl = slice(ec * 512, (ec + 1) * 512)
                        o_ps = bps.tile([128, 2, SC], F32, tag="sc")
                        for t in range(2):
                            att = (AT0, AT1)[t]
                            nc.tensor.matmul(
                                o_ps[:, 0, :], att[:, ssl], WO[:, t, esl],
                                start=(t == 0), stop=(t == 1),
                            )
                        ost = op_pool.tile([128, 512], F32, tag="ost")
                        nc.any.tensor_copy(ost[:], o_ps[:, 0, :])
                        nc.scalar.dma_start(out[ssl, esl], ost[:])

    nc.compile()
    return nc


# host-side column permutations: all rope-even dims first, then all odds
_PERM256 = np.array(
    [64 * h + 2 * i for h in range(4) for i in range(32)]
    + [64 * h + 2 * i + 1 for h in range(4) for i in range(32)]
)
_PERM64 = np.array([2 * i for i in range(32)] + [2 * i + 1 for i in range(32)])

_cache = {}


def make_in_maps(x, cos, sin, wq, wk, wv, wo, n_groups=4):
    S = x.shape[1]
    cos_r = np.ascontiguousarray(np.tile(cos.T, (4, 1)), dtype=np.float16)
    sin_r = np.ascontiguousarray(np.tile(sin.T, (4, 1)), dtype=np.float16)
    tri = np.triu(np.ones((128, 128), dtype=np.float16))
    ident = np.eye(128, dtype=np.float16)
    sel = np.zeros((64, 128), dtype=np.float16)
    sel[0, 0:64] = 1.0
    sel[32, 64:128] = 1.0
    uu, pp_ = np.meshgrid(np.arange(896), np.arange(128))
    ntri = np.where(uu < pp_ + 384, np.float16(-10000.0), np.float16(0.0)).astype(
        np.float16
    )
    xTs = [np.ascontiguousarray(x[b].T.astype(np.float16)) for b in range(x.shape[0])]
    in_maps = []
    for c in range(x.shape[0] * n_groups):
        b, g = divmod(c, n_groups)
        wq_c = np.ascontiguousarray(wq[:, 256 * g + _PERM256].astype(np.float16))
        wk_c = wk[:, 64 * g + _PERM64]
        wv_c = wv[:, 64 * g : 64 * (g + 1)]
        wkv_c = np.ascontiguousarray(
            np.concatenate([wk_c, wv_c], axis=1), dtype=np.float16
        )
        wo_c = np.ascontiguousarray(wo[256 * g : 256 * (g + 1), :].astype(np.float16))
        in_maps.append(
            {
                "xT": xTs[b],
                "wq": wq_c,
                "wkv": wkv_c,
                "wo": wo_c,
                "cosr": cos_r,
                "sinr": sin_r,
                "tri": tri,
                "ident": ident,
                "sel": sel,
                "ntri": ntri,
            }
        )
    return in_maps


def kernel(x, cos, sin, mask, wq, wk, wv, wo):
    x = np.asarray(x, dtype=np.float32)
    cos = np.asarray(cos, dtype=np.float32)
    sin = np.asarray(sin, dtype=np.float32)
    wq = np.asarray(wq, dtype=np.float32)
    wk = np.asarray(wk, dtype=np.float32)
    wv = np.asarray(wv, dtype=np.float32)
    wo = np.asarray(wo, dtype=np.float32)

    if "nc" not in _cache:
        _cache["nc"] = build_nc(S=x.shape[1], n_cores=8)
    nc = _cache["nc"]
    in_maps = make_in_maps(x, cos, sin, wq, wk, wv, wo)
    res = run_bass_kernel_spmd(nc, in_maps, list(range(8)))
    _cache["last"] = res
    outs = [r["out"] for r in res.results]
    final = np.stack(
        [outs[0] + outs[1] + outs[2] + outs[3], outs[4] + outs[5] + outs[6] + outs[7]],
        axis=0,
    )
    return final.astype(np.float32)
